# revision 44
# baseline (speedup 1.0000x reference)
"""Multi-head attention (B=4, N=2048, D=1024, H=16) on 8 Trainium2 cores.

Sharding: core = (batch b, head-group hg) -> 4 batches x 2 groups of 8 heads.

v2 design (vs the M=65 O^T-PV baseline):
  - All matmul operands in bf16 (x, W, Q^T, K^T, V, P, O, W_proj): same PE
    rate as f32r, half the DMA/SBUF. Measured end-to-end rel err ~4e-3.
  - Transposed PV: stationary = P[t, q-tile 128], moving = V[t, 64f + ones
    col] -> out O[q 128, 65] accumulated over 16 token chunks. 65 output
    rows/matmul instead of 512 -> PV cost halves (133k vs 262k PE rows).
    The denominator (ones col) lands per-partition, so softmax
    normalization is a reciprocal + per-partition tensor_scalar_mul on DVE
    (no gpsimd broadcasts).
  - O[q, f] is PE-transposed (bf16 identity) into O^T[f, q] tiles feeding
    the output projection, which is unchanged in orientation.
  - Schedule: per unit (i-block x head-pair), phase A streams the 16 j-chunk
    score+exp tiles of head A on ACT (the critical engine, ~267us of exp;
    ap=1024 amortizes its 185ns access latency) while the PE interleaves
    extras between score matmuls: K/Q chunks for future units (split into
    4-matmul halves, with emit-before-slot deadlines where a unit's own
    scores consume them), the V pass (unit 0), transposed PV of head B of
    the previous unit plus its transposes, and the i0 output projection
    (late units). Phase B mirrors for head B and runs PV of head A of the
    SAME unit (p(u,A) completes exactly at B start), so only two 32KB p
    tiles are ever live. Extras finish a slot early so phase-boundary
    scores are never queued behind leftovers. x^T stays fully resident
    (4 x 8KB) and a warm-up matmul burst ramps the PE p-state during the
    head DMAs (sem-waits reset the clock to 0.65/1.2GHz for ~3us).
  - PSUM: 3x2-bank score/chunk buffers + 2 banks PV-acc/transpose = 8.
    Tail: per-q-tile PV -> transpose -> projection chains, lag-1.

Host feeds x^T, W slices pre-cast to bf16; output partials are summed on
host (b_proj added there too, matching the linearity of the projection).
"""

import sys

if "/opt/trn_rl_repo" not in sys.path:
    sys.path.insert(0, "/opt/trn_rl_repo")

from contextlib import ExitStack

import numpy as np

B, N, D, H = 4, 2048, 1024, 16
HG = 2                 # head groups (tensor parallel)
NCORES = B * HG        # 8
DH = D // HG           # 512 features per group = 8 heads * 64
P = 128
KC = D // P            # 8 contraction chunks over d_model
CP = DH // P           # 4 head pairs (feature 128-chunks) per core
NT = N // 512          # 4 token 512-chunks
TJ = N // P            # 16 token 128-chunks (attention j axis)
IB = 1024              # i-block (exp free-dim)
NI = N // IB           # 2
QT = IB // P           # 8 query 128-tiles per i-block
SCALE = (D // H) ** -0.5

_cached = {}


def _build():
    import concourse.mybir as mybir
    import concourse.tile as tile
    from concourse import bacc
    from concourse.masks import make_identity

    f32 = mybir.dt.float32
    bf16 = mybir.dt.bfloat16
    AF = mybir.ActivationFunctionType

    nc = bacc.Bacc("TRN2", target_bir_lowering=False, debug=False,
                   enable_asserts=False)

    xt = nc.dram_tensor("xt", (D, N), bf16, kind="ExternalInput").ap()
    wq = nc.dram_tensor("wq", (D, DH), bf16, kind="ExternalInput").ap()
    wk = nc.dram_tensor("wk", (D, DH), bf16, kind="ExternalInput").ap()
    wv = nc.dram_tensor("wv", (D, DH), bf16, kind="ExternalInput").ap()
    wp = nc.dram_tensor("wp", (DH, D), bf16, kind="ExternalInput").ap()
    # m0 column slices of wk/wq, host-prepacked partition-major so the
    # head-critical DMAs run with 2KB contiguous runs (the sliced
    # rearrange view only manages 256B runs = half bandwidth)
    wk0 = nc.dram_tensor("wk0", (P, KC, P), bf16, kind="ExternalInput").ap()
    wq0 = nc.dram_tensor("wq0", (P, KC, P), bf16, kind="ExternalInput").ap()
    bq = nc.dram_tensor("bq", (1, DH), f32, kind="ExternalInput").ap()
    bk = nc.dram_tensor("bk", (1, DH), f32, kind="ExternalInput").ap()
    bv = nc.dram_tensor("bv", (1, DH), bf16, kind="ExternalInput").ap()
    y = nc.dram_tensor("y", (N, D), bf16, kind="ExternalOutput").ap()

    xt_r = xt.rearrange("(ko p) t -> p ko t", p=P)
    wq_r = wq.rearrange("(ko p) m -> p ko m", p=P)
    wk_r = wk.rearrange("(ko p) m -> p ko m", p=P)
    wv_r = wv.rearrange("(ko p) m -> p ko m", p=P)

    with tile.TileContext(nc) as tc, ExitStack() as ctx:
        const = ctx.enter_context(tc.tile_pool(name="const", bufs=1))
        persist = ctx.enter_context(tc.tile_pool(name="persist", bufs=1))
        xpool = ctx.enter_context(tc.tile_pool(name="xs", bufs=4))
        wqp = ctx.enter_context(tc.tile_pool(name="wqp", bufs=1))
        wkp = ctx.enter_context(tc.tile_pool(name="wkp", bufs=1))
        wvp = ctx.enter_context(tc.tile_pool(name="wvp", bufs=1))
        ppool = ctx.enter_context(tc.tile_pool(name="pp", bufs=2))
        ospool = ctx.enter_context(tc.tile_pool(name="os", bufs=2))
        drpool = ctx.enter_context(tc.tile_pool(name="dr", bufs=4))
        ypool = ctx.enter_context(tc.tile_pool(name="yb", bufs=3))
        psp = ctx.enter_context(tc.tile_pool(name="psp", bufs=2, space="PSUM"))

        # ---- constants ----
        ones_f32 = const.tile([1, 16], f32)
        nc.vector.memset(ones_f32[:], 1.0)
        ident = const.tile([P, P], bf16)
        make_identity(nc, ident[:])
        bq_sb = const.tile([P, 1, CP], f32)
        bk_sb = const.tile([P, 1, CP], f32)
        bv_sb = const.tile([1, DH], bf16)
        bv_bc = const.tile([P, DH], bf16)

        def load_biases():
            nc.sync.dma_start(bq_sb[:],
                              bq.rearrange("a (mo p) -> p a mo", p=P))
            nc.sync.dma_start(bk_sb[:],
                              bk.rearrange("a (mo p) -> p a mo", p=P))
            nc.sync.dma_start(bv_sb[:], bv)
            nc.gpsimd.partition_broadcast(bv_bc[:], bv_sb[:])
        # preload exp table (no cost in sim; needed on HW)
        dummy = const.tile([1, 16], f32)
        nc.scalar.activation(dummy[:], ones_f32[0:1, 0:16], AF.Exp)
        warm = const.tile([P, 512], bf16)
        nc.vector.memset(warm[:], 0.0)

        qt = persist.tile([P, CP, N], bf16)       # Q^T  [128, 4, 2048]
        kt = persist.tile([P, CP, N], bf16)       # K^T  [128, 4, 2048]
        # V in [token, feature] layout, 65-wide head slots (col 64 = ones
        # -> PV row 64... here PV col 64 = softmax denominator)
        vsb = persist.tile([P, TJ, H // HG, 65], bf16)
        nc.vector.memset(vsb[:, :, :, 64:65], 1.0)
        ot = persist.tile([P, CP, N], bf16)       # O^T  [128, 4, 2048]

        # ---- weights ----
        wq_sb = wqp.tile([P, KC, DH], bf16)
        wk_sb = wkp.tile([P, KC, DH], bf16)
        wv_sb = wvp.tile([P, KC, DH], bf16)
        # m0 column slices first: k(0,0)/q(0,0) are on the first-exp
        # critical path and need only columns 0:128
        nc.sync.dma_start(wk_sb[:, :, 0:P], wk0)
        wp_sb = None  # loaded later into freed weight bytes

        # ---- x streaming ----
        xtiles = {}

        def load_x(n, halves=False):
            xt_t = xpool.tile([P, KC, 512], bf16, tag="xt", name="xt_t")
            if halves:
                nc.sync.dma_start(xt_t[:, 0:4, :],
                                  xt_r[:, 0:4, n * 512:(n + 1) * 512])
                nc.sync.dma_start(xt_t[:, 4:KC, :],
                                  xt_r[:, 4:KC, n * 512:(n + 1) * 512])
            else:
                nc.sync.dma_start(xt_t[:], xt_r[:, :, n * 512:(n + 1) * 512])
            xtiles[n] = xt_t

        # ---- projection chunk emitters, split into ~0.85us halves so the
        # per-slot PE load between consecutive score matmuls stays smooth ----
        def _proj_halves(w_sb, msl, dst, bias, n, tt=None):
            st = {}

            def half(lo, hi):
                if "pt" not in st:
                    st["pt"] = psp.tile([P, IB], f32, tag="s", bufs=3,
                                        name="pt")
                pt = st["pt"]
                for k in range(lo, hi):
                    if tt is None:
                        nc.tensor.matmul(pt[:, 0:512], w_sb[:, k, msl],
                                         xtiles[n][:, k, :], start=(k == 0),
                                         stop=(k == KC - 1))
                    else:
                        nc.tensor.matmul(pt[:, 0:DH],
                                         xtiles[n][:, k, tt * P:(tt + 1) * P],
                                         w_sb[:, k, :], start=(k == 0),
                                         stop=(k == KC - 1))
                if hi == KC:
                    if tt is None:
                        nc.vector.tensor_scalar_add(dst, pt[:, 0:512], bias)
                    else:
                        nc.vector.tensor_add(
                            dst,
                            pt[:, 0:DH].rearrange("p (h d) -> p h d", d=64),
                            bv_bc[:].rearrange("p (h d) -> p h d", d=64))

            return [lambda: half(0, 4), lambda: half(4, KC)]

        def k_halves(n, m):
            return _proj_halves(wk_sb, slice(m * P, (m + 1) * P),
                                kt[:, m, n * 512:(n + 1) * 512],
                                bk_sb[:, 0, m:m + 1], n)

        def q_halves(n, m):
            return _proj_halves(wq_sb, slice(m * P, (m + 1) * P),
                                qt[:, m, n * 512:(n + 1) * 512],
                                bq_sb[:, 0, m:m + 1], n)

        def v_halves(n, tt):
            return _proj_halves(wv_sb, None, vsb[:, n * 4 + tt, :, 0:64],
                                None, n, tt=tt)

        def k_halves_dl(n, m):
            # both halves must land before this unit's scores_a(j=4n)
            return [(f, 4 * n) for f in k_halves(n, m)]

        def k_chunk(n, m):
            for f in k_halves(n, m):
                f()

        def q_chunk(n, m):
            for f in q_halves(n, m):
                f()

        # ---- attention building blocks ----
        def make_pv_group(i, c, h, qtl, p_tile, last):
            # one (q-tile, head) PV group: 16 accumulating matmuls + norm;
            # on the B head additionally transpose the finished O_sb tile.
            def f():
                acc = psp.tile([P, 512], f32, tag="acc", bufs=2, name="acc")
                hh = 2 * c + h
                qsl = slice(qtl * P, (qtl + 1) * P)
                for j in range(TJ):
                    nc.tensor.matmul(acc[:, 0:65], p_tile[:, j, qsl],
                                     vsb[:, j, hh, :], start=(j == 0),
                                     stop=(j == TJ - 1))
                dr = drpool.tile([P, 1], f32, tag="dr", name="dr")
                nc.vector.reciprocal(dr[:], acc[:, 64:65])
                osb = osb_tiles[(i, c)]
                nc.vector.tensor_scalar_mul(
                    osb[:, qtl, h * 64:(h + 1) * 64], acc[:, 0:64], dr[:])
                if last:
                    tp = psp.tile([P, P], bf16, tag="acc", bufs=2, name="tp")
                    nc.tensor.matmul(tp[:], osb[:, qtl, :], ident[:],
                                     is_transpose=True)
                    nc.vector.tensor_copy(
                        ot[:, c, i * IB + qtl * P:i * IB + (qtl + 1) * P],
                        tp[:])
            return f

        def make_proj(tt):
            # one output-projection unit: y[tt*128:+128, :]
            def f():
                yp = psp.tile([P, IB], f32, tag="s", bufs=3, name="yp")
                for o in range(2):
                    for cc in range(CP):
                        nc.tensor.matmul(
                            yp[:, o * 512:(o + 1) * 512],
                            ot[:, cc, tt * P:(tt + 1) * P],
                            wp_sb[:, cc, o * 512:(o + 1) * 512],
                            start=(cc == 0), stop=(cc == CP - 1))
                ysb = ypool.tile([P, D], bf16, tag="y", name="ysb")
                nc.vector.tensor_copy(ysb[:], yp[:])
                nc.sync.dma_start(y[tt * P:(tt + 1) * P, :], ysb[:])
            return f

        osb_tiles = {}

        def phase(i, c, h, extras):
            """One head-phase: 16 score+exp slots with extras interleaved.

            Each extra is a closure or a (closure, deadline_slot) pair; a
            deadline pulls the item (and everything queued before it, to
            keep list order stable) in front of that slot's score matmuls.
            """
            norm = [e if isinstance(e, tuple) else (e, None) for e in extras]
            p_tile = ppool.tile([P, TJ, IB], bf16, tag="p", name="p_t")
            ne = len(norm)
            base = 64 * h
            done = 0
            for j in range(TJ):
                while done < ne and norm[done][1] is not None                         and norm[done][1] <= j:
                    norm[done][0]()
                    done += 1
                s_t = psp.tile([P, IB], f32, tag="s", bufs=3, name="s_t")
                ksl = slice(j * P, (j + 1) * P)
                for iq in range(2):
                    isl = slice(i * IB + iq * 512, i * IB + (iq + 1) * 512)
                    osl = slice(iq * 512, (iq + 1) * 512)
                    nc.tensor.matmul(s_t[:, osl], kt[base:base + 64, c, ksl],
                                     qt[base:base + 64, c, isl],
                                     start=True, stop=True)
                nc.scalar.activation(p_tile[:, j, :], s_t[:], AF.Exp,
                                     scale=SCALE)
                # finish extras a slot early so the next phase's first
                # scores are not queued behind leftover extras
                want = min(ne, (j + 1) * ne // (TJ - 1))
                while done < want:
                    norm[done][0]()
                    done += 1
            while done < ne:
                norm[done][0]()
                done += 1
            return p_tile

        # =========================== schedule ===========================
        units = [(i, c) for i in range(NI) for c in range(CP)]
        p_tiles = {}   # (unit_idx, h) -> p tile

        def pv_extras(u, h, last):
            i, c = units[u]
            # p_tiles[(u, h)] is looked up at emission time: for the
            # (u7, A) groups scheduled inside u7's own B phase, the tile
            # does not exist yet when the extras list is built.
            return [
                (lambda qtl=qtl: make_pv_group(i, c, h, qtl,
                                               p_tiles[(u, h)], last)())
                for qtl in range(QT)
            ]

        # ---- head: x stream + K(.,m0) + Q(n0/n1,m0); V starts inside u0
        # (keeping the first scores off the V/wv DMA critical path) ----
        load_x(0, halves=True)
        nc.sync.dma_start(wq_sb[:, :, 0:P], wq0)
        load_biases()
        # ramp the PE p-state to full clock while the first DMAs fly; the
        # cost model only reaches 2.4GHz after ~3us of continuous execution
        wp_ps = psp.tile([P, 512], f32, tag="acc", bufs=2, name="wp_ps")
        for _ in range(8):
            nc.tensor.matmul(wp_ps[:], warm[:, 0:P], warm[:], start=True,
                             stop=True)
        k_chunk(0, 0)
        q_chunk(0, 0)
        load_x(1, halves=True)
        k_chunk(1, 0)
        q_chunk(1, 0)
        load_x(2, halves=True)
        load_x(3, halves=True)
        nc.sync.dma_start(wv_sb[:, 0:4, :], wv_r[:, 0:4, :])
        nc.sync.dma_start(wv_sb[:, 4:KC, :], wv_r[:, 4:KC, :])
        nc.sync.dma_start(wk_sb[:, :, P:DH], wk_r[:, :, P:DH])
        nc.sync.dma_start(wq_sb[:, :, P:DH], wq_r[:, :, P:DH])

        proj_q = []   # i0 projection units, consumed as fillers in u5/u6

        # Steady state: PV of head A runs in the unit's own B phase
        # (p(u,A) completes exactly as B starts); PV of head B (+the
        # transposes, which need both heads' norms) runs in the next
        # unit's A phase. Only 2 p tiles are ever live.
        for u, (i, c) in enumerate(units):
            if (i, c) not in osb_tiles:
                osb_tiles[(i, c)] = ospool.tile([P, QT, P], bf16, tag="osb",
                                                name="osb")
            # ---------- extras for phase A ----------
            def weave(heavy, light):
                # round-robin merge keeping each list's internal order
                out, hi, li = [], 0, 0
                while hi < len(heavy) or li < len(light):
                    if hi < len(heavy):
                        out.append(heavy[hi]); hi += 1
                    if li < len(light):
                        out.append(light[li]); li += 1
                return out

            ea = []
            if u == 0:
                # k(2,0)/k(3,0) gate this unit's own scores j8/j12
                ea += k_halves_dl(2, 0) + k_halves_dl(3, 0)
                for tt in range(4):
                    ea += v_halves(3, tt)
                for tt in range(4):
                    ea += v_halves(2, tt)
            elif u in (1, 2, 3):
                heavy = []
                if u == 1:
                    heavy += k_halves_dl(1, 1)
                heavy += k_halves_dl(2, c)
                heavy += k_halves_dl(3, c)
                ea = weave(heavy, pv_extras(u - 1, 1, last=True))
            else:
                light = pv_extras(u - 1, 1, last=True)
                heavy = []
                if u == 4:
                    heavy += q_halves(2, 1) + q_halves(3, 1)
                elif u == 5:
                    heavy += q_halves(2, 2) + q_halves(3, 2)
                    heavy += proj_q[0:1]
                elif u == 6:
                    heavy += q_halves(2, 3) + q_halves(3, 3)
                    heavy += proj_q[2:5]
                ea = weave(heavy, light)
            # ---------- extras for phase B ----------
            if u == 0:
                eb = []
                for tt in range(4):
                    eb += v_halves(1, tt)
                for tt in range(4):
                    eb += v_halves(0, tt)
                eb += k_halves(0, 1) + q_halves(0, 1) + q_halves(1, 1)
                eb += pv_extras(0, 0, last=False)
            elif u in (1, 2):
                heavy = k_halves(0, c + 1) + q_halves(0, c + 1)
                heavy += k_halves(1, c + 1) + q_halves(1, c + 1)
                eb = weave(heavy, pv_extras(u, 0, last=False))
            elif u == 3:
                eb = weave(q_halves(2, 0) + q_halves(3, 0),
                           pv_extras(u, 0, last=False))
            else:
                heavy = []
                if u == 5:
                    heavy += proj_q[1:2]
                elif u == 6:
                    heavy += proj_q[5:8]
                eb = weave(heavy, pv_extras(u, 0, last=False))

            p_tiles[(u, 0)] = phase(i, c, 0, ea)
            p_tiles[(u, 1)] = phase(i, c, 1, eb)

            if u == 3:
                # W_proj arrives before the first proj filler (u5-A)
                p2 = ctx.enter_context(tc.tile_pool(name="p2", bufs=1))
                wp_sb = p2.tile([P, CP, D], bf16)
                nc.sync.dma_start(wp_sb[:],
                                  wp.rearrange("(c p) o -> p c o", p=P))
                proj_q = [make_proj(tt) for tt in range(QT)]

        # ---------------- tail: per-qt chains with lag-1 stagger so
        # proj(qt) overlaps the PV/transpose of qt+1 ----------------
        tail_pv = pv_extras(7, 1, last=True)
        tail_pv[0]()
        for qtl in range(QT):
            if qtl + 1 < QT:
                tail_pv[qtl + 1]()
            make_proj(QT + qtl)()

    nc.compile()
    return nc


def _get_nc():
    if "nc" not in _cached:
        _cached["nc"] = _build()
    return _cached["nc"]


def kernel(x, W_qkv, b_qkv, W_proj, b_proj):
    import ml_dtypes
    from concourse.bass_utils import run_bass_kernel_spmd

    bf = ml_dtypes.bfloat16
    x = np.asarray(x, dtype=np.float32)
    W_qkv = np.asarray(W_qkv, dtype=np.float32)
    b_qkv = np.asarray(b_qkv, dtype=np.float32)
    W_proj = np.asarray(W_proj, dtype=np.float32)
    b_proj = np.asarray(b_proj, dtype=np.float32)

    in_maps = []
    for core in range(NCORES):
        b, hg = divmod(core, HG)
        hs = slice(DH * hg, DH * (hg + 1))
        wq_np = np.ascontiguousarray(W_qkv[:, hs]).astype(bf)
        wk_np = np.ascontiguousarray(
            W_qkv[:, D + DH * hg:D + DH * (hg + 1)]).astype(bf)
        in_maps.append({
            "xt": np.ascontiguousarray(x[b].T).astype(bf),
            "wq": wq_np,
            "wk0": np.ascontiguousarray(
                wk_np[:, 0:P].reshape(KC, P, P).transpose(1, 0, 2)),
            "wq0": np.ascontiguousarray(
                wq_np[:, 0:P].reshape(KC, P, P).transpose(1, 0, 2)),
            "wk": wk_np,
            "wv": np.ascontiguousarray(W_qkv[:, 2 * D + DH * hg:2 * D + DH * (hg + 1)]).astype(bf),
            "wp": np.ascontiguousarray(W_proj[hs, :]).astype(bf),
            "bq": b_qkv[hs][None, :],
            "bk": b_qkv[D + DH * hg:D + DH * (hg + 1)][None, :],
            "bv": b_qkv[2 * D + DH * hg:2 * D + DH * (hg + 1)][None, :].astype(bf),
        })

    nc = _get_nc()
    res = run_bass_kernel_spmd(nc, in_maps, core_ids=list(range(NCORES)))
    out = np.empty((B, N, D), dtype=np.float32)
    for b in range(B):
        out[b] = (res.results[2 * b]["y"].astype(np.float32)
                  + res.results[2 * b + 1]["y"].astype(np.float32) + b_proj)
    return out


# revision 47
# speedup vs baseline: 1.0032x; 1.0032x over previous
"""Multi-head attention (B=4, N=2048, D=1024, H=16) on 8 Trainium2 cores.

Sharding: core = (batch b, head-group hg) -> 4 batches x 2 groups of 8 heads.

v2 design (vs the M=65 O^T-PV baseline):
  - All matmul operands in bf16 (x, W, Q^T, K^T, V, P, O, W_proj): same PE
    rate as f32r, half the DMA/SBUF. Measured end-to-end rel err ~4e-3.
  - Transposed PV: stationary = P[t, q-tile 128], moving = V[t, 64f + ones
    col] -> out O[q 128, 65] accumulated over 16 token chunks. 65 output
    rows/matmul instead of 512 -> PV cost halves (133k vs 262k PE rows).
    The denominator (ones col) lands per-partition, so softmax
    normalization is a reciprocal + per-partition tensor_scalar_mul on DVE
    (no gpsimd broadcasts).
  - O[q, f] is PE-transposed (bf16 identity) into O^T[f, q] tiles feeding
    the output projection, which is unchanged in orientation.
  - Schedule: per unit (i-block x head-pair), phase A streams the 16 j-chunk
    score+exp tiles of head A on ACT (the critical engine, ~267us of exp;
    ap=1024 amortizes its 185ns access latency) while the PE interleaves
    extras between score matmuls: K/Q chunks for future units (split into
    4-matmul halves, with emit-before-slot deadlines where a unit's own
    scores consume them), the V pass (unit 0), transposed PV of head B of
    the previous unit plus its transposes, and the i0 output projection
    (late units). Phase B mirrors for head B and runs PV of head A of the
    SAME unit (p(u,A) completes exactly at B start), so only two 32KB p
    tiles are ever live. Extras finish a slot early so phase-boundary
    scores are never queued behind leftovers. x^T stays fully resident
    (4 x 8KB) and a warm-up matmul burst ramps the PE p-state during the
    head DMAs (sem-waits reset the clock to 0.65/1.2GHz for ~3us).
  - PSUM: 3x2-bank score/chunk buffers + 2 banks PV-acc/transpose = 8.
    Tail: per-q-tile PV -> transpose -> projection chains, lag-1.

Host feeds x^T, W slices pre-cast to bf16; output partials are summed on
host (b_proj added there too, matching the linearity of the projection).
"""

import sys

if "/opt/trn_rl_repo" not in sys.path:
    sys.path.insert(0, "/opt/trn_rl_repo")

from contextlib import ExitStack

import numpy as np

B, N, D, H = 4, 2048, 1024, 16
HG = 2                 # head groups (tensor parallel)
NCORES = B * HG        # 8
DH = D // HG           # 512 features per group = 8 heads * 64
P = 128
KC = D // P            # 8 contraction chunks over d_model
CP = DH // P           # 4 head pairs (feature 128-chunks) per core
NT = N // 512          # 4 token 512-chunks
TJ = N // P            # 16 token 128-chunks (attention j axis)
IB = 1024              # i-block (exp free-dim)
NI = N // IB           # 2
QT = IB // P           # 8 query 128-tiles per i-block
SCALE = (D // H) ** -0.5

_cached = {}


def _build():
    import concourse.mybir as mybir
    import concourse.tile as tile
    from concourse import bacc
    from concourse.masks import make_identity

    f32 = mybir.dt.float32
    bf16 = mybir.dt.bfloat16
    AF = mybir.ActivationFunctionType

    nc = bacc.Bacc("TRN2", target_bir_lowering=False, debug=False,
                   enable_asserts=False)

    xt = nc.dram_tensor("xt", (D, N), bf16, kind="ExternalInput").ap()
    wq = nc.dram_tensor("wq", (D, DH), bf16, kind="ExternalInput").ap()
    wk = nc.dram_tensor("wk", (D, DH), bf16, kind="ExternalInput").ap()
    wv = nc.dram_tensor("wv", (D, DH), bf16, kind="ExternalInput").ap()
    wp = nc.dram_tensor("wp", (DH, D), bf16, kind="ExternalInput").ap()
    # m0 column slices of wk and wq, host-prepacked partition-major into
    # ONE tensor: transfers of 2-4KB/partition all cost the same ~1.46us
    # floor, so one merged transfer removes a whole slot from the
    # DMA-serial chain that gates the PE stream start
    wkq0 = nc.dram_tensor("wkq0", (P, KC, 2 * P), bf16,
                          kind="ExternalInput").ap()
    bq = nc.dram_tensor("bq", (1, DH), f32, kind="ExternalInput").ap()
    bk = nc.dram_tensor("bk", (1, DH), f32, kind="ExternalInput").ap()
    bv = nc.dram_tensor("bv", (1, DH), bf16, kind="ExternalInput").ap()
    y = nc.dram_tensor("y", (N, D), bf16, kind="ExternalOutput").ap()

    xt_r = xt.rearrange("(ko p) t -> p ko t", p=P)
    wq_r = wq.rearrange("(ko p) m -> p ko m", p=P)
    wk_r = wk.rearrange("(ko p) m -> p ko m", p=P)
    wv_r = wv.rearrange("(ko p) m -> p ko m", p=P)

    with tile.TileContext(nc) as tc, ExitStack() as ctx:
        const = ctx.enter_context(tc.tile_pool(name="const", bufs=1))
        persist = ctx.enter_context(tc.tile_pool(name="persist", bufs=1))
        xpool = ctx.enter_context(tc.tile_pool(name="xs", bufs=4))
        wqp = ctx.enter_context(tc.tile_pool(name="wqp", bufs=1))
        wkp = ctx.enter_context(tc.tile_pool(name="wkp", bufs=1))
        wvp = ctx.enter_context(tc.tile_pool(name="wvp", bufs=1))
        ppool = ctx.enter_context(tc.tile_pool(name="pp", bufs=2))
        ospool = ctx.enter_context(tc.tile_pool(name="os", bufs=2))
        drpool = ctx.enter_context(tc.tile_pool(name="dr", bufs=4))
        ypool = ctx.enter_context(tc.tile_pool(name="yb", bufs=3))
        psp = ctx.enter_context(tc.tile_pool(name="psp", bufs=2, space="PSUM"))

        # ---- constants ----
        ones_f32 = const.tile([1, 16], f32)
        nc.vector.memset(ones_f32[:], 1.0)
        ident = const.tile([P, P], bf16)
        make_identity(nc, ident[:])
        bq_sb = const.tile([P, 1, CP], f32)
        bk_sb = const.tile([P, 1, CP], f32)
        bv_sb = const.tile([1, DH], bf16)
        bv_bc = const.tile([P, DH], bf16)

        def load_biases():
            nc.sync.dma_start(bq_sb[:],
                              bq.rearrange("a (mo p) -> p a mo", p=P))
            nc.sync.dma_start(bk_sb[:],
                              bk.rearrange("a (mo p) -> p a mo", p=P))
            nc.sync.dma_start(bv_sb[:], bv)
            nc.gpsimd.partition_broadcast(bv_bc[:], bv_sb[:])
        # preload exp table (no cost in sim; needed on HW)
        dummy = const.tile([1, 16], f32)
        nc.scalar.activation(dummy[:], ones_f32[0:1, 0:16], AF.Exp)
        warm = const.tile([P, 512], bf16)
        nc.vector.memset(warm[:], 0.0)

        qt = persist.tile([P, CP, N], bf16)       # Q^T  [128, 4, 2048]
        kt = persist.tile([P, CP, N], bf16)       # K^T  [128, 4, 2048]
        # V in [token, feature] layout, 65-wide head slots (col 64 = ones
        # -> PV row 64... here PV col 64 = softmax denominator)
        vsb = persist.tile([P, TJ, H // HG, 65], bf16)
        nc.vector.memset(vsb[:, :, :, 64:65], 1.0)
        ot = persist.tile([P, CP, N], bf16)       # O^T  [128, 4, 2048]

        # ---- weights ----
        # m1..m3 columns only; the m0 slices live in wkq0_sb
        wq_sb = wqp.tile([P, KC, DH - P], bf16)
        wk_sb = wkp.tile([P, KC, DH - P], bf16)
        wv_sb = wvp.tile([P, KC, DH], bf16)
        # m0 column slices first: k(0,0)/q(0,0) are on the first-exp
        # critical path and need only columns 0:128 of wk/wq
        wkq0_sb = const.tile([P, KC, 2 * P], bf16)
        nc.sync.dma_start(wkq0_sb[:], wkq0)
        wp_sb = None  # loaded later into freed weight bytes

        # ---- x streaming ----
        xtiles = {}

        def load_x(n, halves=False):
            xt_t = xpool.tile([P, KC, 512], bf16, tag="xt", name="xt_t")
            if halves:
                nc.sync.dma_start(xt_t[:, 0:4, :],
                                  xt_r[:, 0:4, n * 512:(n + 1) * 512])
                nc.sync.dma_start(xt_t[:, 4:KC, :],
                                  xt_r[:, 4:KC, n * 512:(n + 1) * 512])
            else:
                nc.sync.dma_start(xt_t[:], xt_r[:, :, n * 512:(n + 1) * 512])
            xtiles[n] = xt_t

        # ---- projection chunk emitters, split into ~0.85us halves so the
        # per-slot PE load between consecutive score matmuls stays smooth ----
        def _proj_halves(w_sb, msl, dst, bias, n, tt=None):
            st = {}

            def half(lo, hi):
                if "pt" not in st:
                    st["pt"] = psp.tile([P, IB], f32, tag="s", bufs=3,
                                        name="pt")
                pt = st["pt"]
                for k in range(lo, hi):
                    if tt is None:
                        nc.tensor.matmul(pt[:, 0:512], w_sb[:, k, msl],
                                         xtiles[n][:, k, :], start=(k == 0),
                                         stop=(k == KC - 1))
                    else:
                        nc.tensor.matmul(pt[:, 0:DH],
                                         xtiles[n][:, k, tt * P:(tt + 1) * P],
                                         w_sb[:, k, :], start=(k == 0),
                                         stop=(k == KC - 1))
                if hi == KC:
                    if tt is None:
                        nc.vector.tensor_scalar_add(dst, pt[:, 0:512], bias)
                    else:
                        nc.vector.tensor_add(
                            dst,
                            pt[:, 0:DH].rearrange("p (h d) -> p h d", d=64),
                            bv_bc[:].rearrange("p (h d) -> p h d", d=64))

            return [lambda: half(0, 4), lambda: half(4, KC)]

        def k_halves(n, m):
            w, sl = ((wkq0_sb, slice(0, P)) if m == 0
                     else (wk_sb, slice((m - 1) * P, m * P)))
            return _proj_halves(w, sl, kt[:, m, n * 512:(n + 1) * 512],
                                bk_sb[:, 0, m:m + 1], n)

        def q_halves(n, m):
            w, sl = ((wkq0_sb, slice(P, 2 * P)) if m == 0
                     else (wq_sb, slice((m - 1) * P, m * P)))
            return _proj_halves(w, sl, qt[:, m, n * 512:(n + 1) * 512],
                                bq_sb[:, 0, m:m + 1], n)

        def v_halves(n, tt):
            return _proj_halves(wv_sb, None, vsb[:, n * 4 + tt, :, 0:64],
                                None, n, tt=tt)

        def k_halves_dl(n, m):
            # both halves must land before this unit's scores_a(j=4n)
            return [(f, 4 * n) for f in k_halves(n, m)]

        def k_chunk(n, m):
            for f in k_halves(n, m):
                f()

        def q_chunk(n, m):
            for f in q_halves(n, m):
                f()

        # ---- attention building blocks ----
        def make_pv_group(i, c, h, qtl, p_tile, last):
            # one (q-tile, head) PV group: 16 accumulating matmuls + norm;
            # on the B head additionally transpose the finished O_sb tile.
            def f():
                acc = psp.tile([P, 512], f32, tag="acc", bufs=2, name="acc")
                hh = 2 * c + h
                qsl = slice(qtl * P, (qtl + 1) * P)
                for j in range(TJ):
                    nc.tensor.matmul(acc[:, 0:65], p_tile[:, j, qsl],
                                     vsb[:, j, hh, :], start=(j == 0),
                                     stop=(j == TJ - 1))
                dr = drpool.tile([P, 1], f32, tag="dr", name="dr")
                nc.vector.reciprocal(dr[:], acc[:, 64:65])
                osb = osb_tiles[(i, c)]
                nc.vector.tensor_scalar_mul(
                    osb[:, qtl, h * 64:(h + 1) * 64], acc[:, 0:64], dr[:])
                if last:
                    tp = psp.tile([P, P], bf16, tag="acc", bufs=2, name="tp")
                    nc.tensor.matmul(tp[:], osb[:, qtl, :], ident[:],
                                     is_transpose=True)
                    nc.vector.tensor_copy(
                        ot[:, c, i * IB + qtl * P:i * IB + (qtl + 1) * P],
                        tp[:])
            return f

        def make_proj(tt):
            # one output-projection unit: y[tt*128:+128, :]
            def f():
                yp = psp.tile([P, IB], f32, tag="s", bufs=3, name="yp")
                for o in range(2):
                    for cc in range(CP):
                        nc.tensor.matmul(
                            yp[:, o * 512:(o + 1) * 512],
                            ot[:, cc, tt * P:(tt + 1) * P],
                            wp_sb[:, cc, o * 512:(o + 1) * 512],
                            start=(cc == 0), stop=(cc == CP - 1))
                ysb = ypool.tile([P, D], bf16, tag="y", name="ysb")
                nc.vector.tensor_copy(ysb[:], yp[:])
                nc.sync.dma_start(y[tt * P:(tt + 1) * P, :], ysb[:])
            return f

        osb_tiles = {}

        def phase(i, c, h, extras):
            """One head-phase: 16 score+exp slots with extras interleaved.

            Each extra is a closure or a (closure, deadline_slot) pair; a
            deadline pulls the item (and everything queued before it, to
            keep list order stable) in front of that slot's score matmuls.
            """
            norm = [e if isinstance(e, tuple) else (e, None) for e in extras]
            p_tile = ppool.tile([P, TJ, IB], bf16, tag="p", name="p_t")
            ne = len(norm)
            base = 64 * h
            done = 0
            for j in range(TJ):
                while done < ne and norm[done][1] is not None                         and norm[done][1] <= j:
                    norm[done][0]()
                    done += 1
                s_t = psp.tile([P, IB], f32, tag="s", bufs=3, name="s_t")
                ksl = slice(j * P, (j + 1) * P)
                for iq in range(2):
                    isl = slice(i * IB + iq * 512, i * IB + (iq + 1) * 512)
                    osl = slice(iq * 512, (iq + 1) * 512)
                    nc.tensor.matmul(s_t[:, osl], kt[base:base + 64, c, ksl],
                                     qt[base:base + 64, c, isl],
                                     start=True, stop=True)
                nc.scalar.activation(p_tile[:, j, :], s_t[:], AF.Exp,
                                     scale=SCALE)
                # finish extras a slot early so the next phase's first
                # scores are not queued behind leftover extras
                want = min(ne, (j + 1) * ne // (TJ - 1))
                while done < want:
                    norm[done][0]()
                    done += 1
            while done < ne:
                norm[done][0]()
                done += 1
            return p_tile

        # =========================== schedule ===========================
        units = [(i, c) for i in range(NI) for c in range(CP)]
        p_tiles = {}   # (unit_idx, h) -> p tile

        def pv_extras(u, h, last):
            i, c = units[u]
            # p_tiles[(u, h)] is looked up at emission time: for the
            # (u7, A) groups scheduled inside u7's own B phase, the tile
            # does not exist yet when the extras list is built.
            return [
                (lambda qtl=qtl: make_pv_group(i, c, h, qtl,
                                               p_tiles[(u, h)], last)())
                for qtl in range(QT)
            ]

        # ---- head: x stream + K(.,m0) + Q(n0/n1,m0); V starts inside u0
        # (keeping the first scores off the V/wv DMA critical path) ----
        load_x(0, halves=True)
        load_biases()
        # ramp the PE p-state to full clock while the first DMAs fly; the
        # cost model only reaches 2.4GHz after ~3us of continuous execution
        wp_ps = psp.tile([P, 512], f32, tag="acc", bufs=2, name="wp_ps")
        for _ in range(8):
            nc.tensor.matmul(wp_ps[:], warm[:, 0:P], warm[:], start=True,
                             stop=True)
        k_chunk(0, 0)
        q_chunk(0, 0)
        load_x(1, halves=True)
        k_chunk(1, 0)
        q_chunk(1, 0)
        load_x(2, halves=True)
        load_x(3, halves=True)
        nc.sync.dma_start(wv_sb[:, 0:4, :], wv_r[:, 0:4, :])
        nc.sync.dma_start(wv_sb[:, 4:KC, :], wv_r[:, 4:KC, :])
        nc.sync.dma_start(wk_sb[:], wk_r[:, :, P:DH])
        nc.sync.dma_start(wq_sb[:], wq_r[:, :, P:DH])

        proj_q = []   # i0 projection units, consumed as fillers in u5/u6

        # Steady state: PV of head A runs in the unit's own B phase
        # (p(u,A) completes exactly as B starts); PV of head B (+the
        # transposes, which need both heads' norms) runs in the next
        # unit's A phase. Only 2 p tiles are ever live.
        for u, (i, c) in enumerate(units):
            if (i, c) not in osb_tiles:
                osb_tiles[(i, c)] = ospool.tile([P, QT, P], bf16, tag="osb",
                                                name="osb")
            # ---------- extras for phase A ----------
            def weave(heavy, light):
                # round-robin merge keeping each list's internal order
                out, hi, li = [], 0, 0
                while hi < len(heavy) or li < len(light):
                    if hi < len(heavy):
                        out.append(heavy[hi]); hi += 1
                    if li < len(light):
                        out.append(light[li]); li += 1
                return out

            ea = []
            if u == 0:
                # k(2,0)/k(3,0) gate this unit's own scores j8/j12
                ea += k_halves_dl(2, 0) + k_halves_dl(3, 0)
                for tt in range(4):
                    ea += v_halves(3, tt)
                for tt in range(4):
                    ea += v_halves(2, tt)
            elif u in (1, 2, 3):
                heavy = []
                if u == 1:
                    heavy += k_halves_dl(1, 1)
                heavy += k_halves_dl(2, c)
                heavy += k_halves_dl(3, c)
                ea = weave(heavy, pv_extras(u - 1, 1, last=True))
            else:
                light = pv_extras(u - 1, 1, last=True)
                heavy = []
                if u == 4:
                    heavy += q_halves(2, 1) + q_halves(3, 1)
                elif u == 5:
                    heavy += q_halves(2, 2) + q_halves(3, 2)
                    heavy += proj_q[0:1]
                elif u == 6:
                    heavy += q_halves(2, 3) + q_halves(3, 3)
                    heavy += proj_q[2:5]
                ea = weave(heavy, light)
            # ---------- extras for phase B ----------
            if u == 0:
                eb = []
                for tt in range(4):
                    eb += v_halves(1, tt)
                for tt in range(4):
                    eb += v_halves(0, tt)
                eb += k_halves(0, 1) + q_halves(0, 1) + q_halves(1, 1)
                eb += pv_extras(0, 0, last=False)
            elif u in (1, 2):
                heavy = k_halves(0, c + 1) + q_halves(0, c + 1)
                heavy += k_halves(1, c + 1) + q_halves(1, c + 1)
                eb = weave(heavy, pv_extras(u, 0, last=False))
            elif u == 3:
                eb = weave(q_halves(2, 0) + q_halves(3, 0),
                           pv_extras(u, 0, last=False))
            else:
                heavy = []
                if u == 5:
                    heavy += proj_q[1:2]
                elif u == 6:
                    heavy += proj_q[5:8]
                eb = weave(heavy, pv_extras(u, 0, last=False))

            p_tiles[(u, 0)] = phase(i, c, 0, ea)
            p_tiles[(u, 1)] = phase(i, c, 1, eb)

            if u == 3:
                # W_proj arrives before the first proj filler (u5-A)
                p2 = ctx.enter_context(tc.tile_pool(name="p2", bufs=1))
                wp_sb = p2.tile([P, CP, D], bf16)
                nc.sync.dma_start(wp_sb[:],
                                  wp.rearrange("(c p) o -> p c o", p=P))
                proj_q = [make_proj(tt) for tt in range(QT)]

        # ---------------- tail: per-qt chains with lag-1 stagger so
        # proj(qt) overlaps the PV/transpose of qt+1 ----------------
        tail_pv = pv_extras(7, 1, last=True)
        tail_pv[0]()
        for qtl in range(QT):
            if qtl + 1 < QT:
                tail_pv[qtl + 1]()
            make_proj(QT + qtl)()

    nc.compile()
    return nc


def _get_nc():
    if "nc" not in _cached:
        _cached["nc"] = _build()
    return _cached["nc"]


def kernel(x, W_qkv, b_qkv, W_proj, b_proj):
    import ml_dtypes
    from concourse.bass_utils import run_bass_kernel_spmd

    bf = ml_dtypes.bfloat16
    x = np.asarray(x, dtype=np.float32)
    W_qkv = np.asarray(W_qkv, dtype=np.float32)
    b_qkv = np.asarray(b_qkv, dtype=np.float32)
    W_proj = np.asarray(W_proj, dtype=np.float32)
    b_proj = np.asarray(b_proj, dtype=np.float32)

    in_maps = []
    for core in range(NCORES):
        b, hg = divmod(core, HG)
        hs = slice(DH * hg, DH * (hg + 1))
        wq_np = np.ascontiguousarray(W_qkv[:, hs]).astype(bf)
        wk_np = np.ascontiguousarray(
            W_qkv[:, D + DH * hg:D + DH * (hg + 1)]).astype(bf)
        in_maps.append({
            "xt": np.ascontiguousarray(x[b].T).astype(bf),
            "wq": wq_np,
            "wkq0": np.ascontiguousarray(np.concatenate(
                [wk_np[:, 0:P].reshape(KC, P, P).transpose(1, 0, 2),
                 wq_np[:, 0:P].reshape(KC, P, P).transpose(1, 0, 2)],
                axis=2)),
            "wk": wk_np,
            "wv": np.ascontiguousarray(W_qkv[:, 2 * D + DH * hg:2 * D + DH * (hg + 1)]).astype(bf),
            "wp": np.ascontiguousarray(W_proj[hs, :]).astype(bf),
            "bq": b_qkv[hs][None, :],
            "bk": b_qkv[D + DH * hg:D + DH * (hg + 1)][None, :],
            "bv": b_qkv[2 * D + DH * hg:2 * D + DH * (hg + 1)][None, :].astype(bf),
        })

    nc = _get_nc()
    res = run_bass_kernel_spmd(nc, in_maps, core_ids=list(range(NCORES)))
    out = np.empty((B, N, D), dtype=np.float32)
    for b in range(B):
        out[b] = (res.results[2 * b]["y"].astype(np.float32)
                  + res.results[2 * b + 1]["y"].astype(np.float32) + b_proj)
    return out
